# revision 2
# baseline (speedup 1.0000x reference)
"""ARGenerator TRN2 kernel (v2: single 1024-wide chunk).

Math (per batch row b):
  h1 = relu(x @ W1.T + b1); h2 = relu(h1 @ W2.T + b2)
  mlp = tanh(h2 @ W3.T + b3)
  ar[t] = noise[t] + sum_i c[i] * ar[t-1-i]  (zero-init, t >= 7; 0 for t < 7)
  out = mlp + ar

The AR recurrence is linear time-invariant -> ar = conv(noise_masked, h)
with h the (geometrically decaying) impulse response, truncated at
(nb-1)*128 taps.  The conv becomes nb banded 128x128 Toeplitz matmuls
per output time-tile, fully parallel over time.

Layout strategy (pure data parallel over 8 cores, B_shard = 1024):
  bf16 everywhere (fp8 for x/W1), TRANSPOSED activation layout
  [feature/time on partitions, batch on free dim].  The host
  pre-transposes x and noise shards; output comes back transposed.

v2 pipeline: ONE chunk of CW=1024 (the full shard).  Matmuls write
512-wide PSUM bank halves (HW: matmul cannot cross a PSUM bank), but
the scalar ACT (tanh + per-partition b3 bias) and the DVE add read the
full [128, 1024] tile in ONE instruction, amortizing the (N+352)/1.2ns
fixed overhead: 1147ns per t-tile instead of 2x720.  L1 uses
DoubleRow fp8 matmuls (2 k-tiles per instruction, ~1.44x).  PSUM is
exactly 2 pools x 2 bufs x 2 banks = 8 banks.

Engine budget per t-tile (1024 batch): Tensor 6x216=1296ns,
Scalar 1147, DVE 1147, wire (noise in + out) 1312ns -> wire-paced.
Per-core wire: 0.5MB W1 + 1.1MB consts + 4MB xT + 8MB nT + 8MB out.
"""

import numpy as np
import ml_dtypes

import concourse.bass as bass
import concourse.tile as tile
import concourse.mybir as mybir
from concourse import bacc

F32 = mybir.dt.float32
BF16 = mybir.dt.bfloat16
F8 = mybir.dt.float8e4
BF16_NP = ml_dtypes.bfloat16
F8_NP = ml_dtypes.float8_e4m3
W1_SCALE = 64.0
DR = mybir.MatmulPerfMode.DoubleRow


def impulse_response(c, s_out, tail_tol=1e-4):
    """Return (h, nb) with (nb-1)*128 taps covering the response."""
    AR = len(c)
    c = np.asarray(c, np.float64)
    h = np.zeros(s_out, np.float64)
    h[0] = 1.0
    for j in range(1, s_out):
        acc = 0.0
        for i in range(AR):
            if j - 1 - i >= 0:
                acc += c[i] * h[j - 1 - i]
        h[j] = acc
    L = 128
    while L < s_out and np.abs(h[L:]).sum() > tail_tol:
        L += 128
    # nb = number of 128-wide band blocks per output tile: the in-tile block
    # (j=0) plus one per preceding input tile the L-tap history reaches into.
    return h, L // 128 + 1


def band_blocks(h, nb):
    """Hb [128, nb*128]: block jj (for input-tile offset j = jj - (nb-1))
    has Hb[k_rel, t_rel] = h[t_rel - k_rel - 128*j] (0 <= lag < (nb-1)*128)."""
    L = (nb - 1) * 128
    a = np.arange(128)[:, None]   # k_rel
    b = np.arange(128)[None, :]   # t_rel
    blocks = []
    for jj in range(nb):
        j = jj - (nb - 1)
        lag = b - a - 128 * j
        m = (lag >= 0) & (lag < L)
        blk = np.where(m, np.take(np.pad(h[:L], (0, 1)), np.clip(lag, 0, L)), 0.0)
        blocks.append(blk)
    return np.concatenate(blocks, axis=1)


def host_prepare(W1, b1, W2, b2, W3, b3, ar_coef, S_IN, S_OUT, H):
    """Small device tensors in exactly the SBUF layout used, bf16."""
    n_s = S_IN // 128
    # W1l[p, k, h] = W1[h, k*128 + p]  (lhsT tiles for layer 1)
    W1l = np.ascontiguousarray(
        W1.reshape(H, n_s, 128).transpose(2, 1, 0)
    )
    h, nb = impulse_response(ar_coef, S_OUT)
    return {
        "W1l": (W1l * W1_SCALE).astype(F8_NP),  # fp8, scaled into e4m3 normal range
        "W2l": np.ascontiguousarray(W2.T).astype(BF16_NP),   # [H_in, H_out]
        "W3l": np.ascontiguousarray(W3.T).astype(BF16_NP),   # [H, S_OUT]
        "b1c": np.ascontiguousarray(b1.reshape(H, 1), dtype=np.float32),
        "b2c": np.ascontiguousarray(b2.reshape(H, 1), dtype=np.float32),
        "b3m": np.ascontiguousarray(b3.reshape(S_OUT // 128, 128).T,
                                    dtype=np.float32),       # [128, n_t]
        "Hb": band_blocks(h, nb).astype(BF16_NP),
    }, nb


def build_kernel(B_shard, S_IN, S_OUT, H, nb):
    P = 128
    CW = B_shard                  # 1024: one chunk, the full shard
    HW = CW // 2                  # 512: matmul free width (one PSUM bank)
    assert H == P and CW == 1024 and nb == 2
    n_s = S_IN // P               # 32 input k-tiles
    n_t = S_OUT // P              # 32 output t-tiles

    nc = bacc.Bacc(trn_type="TRN2", target_bir_lowering=False, debug=False)

    xT_d = nc.dram_tensor("xT", [S_IN, CW], F8, kind="ExternalInput").ap()
    nT_d = nc.dram_tensor("nT", [S_OUT, CW], BF16, kind="ExternalInput").ap()
    W1_d = nc.dram_tensor("W1l", [P, n_s, H], F8, kind="ExternalInput").ap()
    W2_d = nc.dram_tensor("W2l", [H, H], BF16, kind="ExternalInput").ap()
    W3_d = nc.dram_tensor("W3l", [H, S_OUT], BF16, kind="ExternalInput").ap()
    b1_d = nc.dram_tensor("b1c", [H, 1], F32, kind="ExternalInput").ap()
    b2_d = nc.dram_tensor("b2c", [H, 1], F32, kind="ExternalInput").ap()
    b3_d = nc.dram_tensor("b3m", [P, n_t], F32, kind="ExternalInput").ap()
    Hb_d = nc.dram_tensor("Hb", [P, nb * P], BF16, kind="ExternalInput").ap()
    out_d = nc.dram_tensor("outT", [S_OUT, CW], BF16,
                           kind="ExternalOutput").ap()

    with tile.TileContext(nc) as tc:
        with tc.tile_pool(name="const", bufs=1) as cpool:
            # W1 split into 4 tiles across both queues: dependency tracking
            # is per-tile, so a monolithic W1 would stall layer-1 k=0 until
            # the whole 0.5MB lands.
            W1t = []
            for i in range(4):
                w = cpool.tile([P, n_s // 4, H], F8, tag=f"w1_{i}")
                (nc.sync if i % 2 == 0 else nc.scalar).dma_start(
                    w[:], W1_d[:, i * (n_s // 4):(i + 1) * (n_s // 4), :])
                W1t.append(w)

            def W1dr(kk):     # k-pair kk -> [128, 2, H] fp8 lhsT
                return W1t[kk // 4][:, 2 * (kk % 4):2 * (kk % 4) + 2, :]

            # scalar queue: small consts (x loads follow on sync+scalar).
            W2s = cpool.tile([H, H], BF16, tag="w2")
            nc.scalar.dma_start(W2s[:], W2_d[:])
            b1s = cpool.tile([H, 1], F32, tag="b1")
            nc.scalar.dma_start(b1s[:], b1_d[:])
            b2s = cpool.tile([H, 1], F32, tag="b2")
            nc.scalar.dma_start(b2s[:], b2_d[:])
            # t-loop consts on the gpsimd queue (idle until stores begin).
            b3s = cpool.tile([P, n_t], F32, tag="b3")
            nc.gpsimd.dma_start(b3s[:], b3_d[:])
            Hbs = cpool.tile([P, nb * P], BF16, tag="hb")
            nc.gpsimd.dma_start(Hbs[:], Hb_d[:])
            W3s = cpool.tile([H, S_OUT], BF16, tag="w3")
            nc.gpsimd.dma_start(W3s[:], W3_d[:])

            with (
                tc.tile_pool(name="warm", bufs=1) as wpool,
                tc.tile_pool(name="xT", bufs=4) as xTp,
                tc.tile_pool(name="nT", bufs=6) as nTp,
                tc.tile_pool(name="act", bufs=2) as actp,
                tc.tile_pool(name="th", bufs=3) as thp,
                tc.tile_pool(name="outT", bufs=3) as outp,
                tc.tile_pool(name="psA", bufs=2, space="PSUM") as psA,
                tc.tile_pool(name="psB", bufs=2, space="PSUM") as psB,
            ):
                # ---- input loads: 4 k-tiles per DMA (512KB), alternating
                # queues so one queue's trigger rate doesn't cap the wire.
                xts = []
                for g in range(n_s // 4):
                    t = xTp.tile([P, 4, CW], F8, tag="xt", name=f"xt{g}")
                    src = xT_d[g * 4 * P:(g + 1) * 4 * P, :].rearrange(
                        "(blk p) f -> p blk f", p=P)
                    (nc.sync if g % 2 == 0 else nc.scalar).dma_start(t[:], src)
                    xts.append(t)

                def xdr(kk, h):   # k-pair kk, batch-half h -> [128,2,512] f8
                    g, j = kk // 2, 2 * (kk % 2)
                    return xts[g][:, j:j + 2, h * HW:(h + 1) * HW]

                ntm = []

                def load_n4(g):
                    t = nTp.tile([P, 4, CW], BF16, tag="nt", name=f"nt{g}")
                    src = nT_d[g * 4 * P:(g + 1) * 4 * P, :].rearrange(
                        "(blk p) f -> p blk f", p=P)
                    nc.sync.dma_start(t[:], src)
                    ntm.append(t)

                def nt(m, h):
                    return ntm[m // 4][:, m % 4, h * HW:(h + 1) * HW]

                for g in range(n_t // 4):
                    load_n4(g)

                # ---- PE warm-up: the HAM clock gate defaults the PE array
                # to 1.2 GHz and only releases 2.4 GHz after ~3.4us of
                # sustained matmul activity; it re-throttles after ~3.4us
                # idle.  The first real matmul cannot start until W1+x
                # arrive (~6us); warm on a zeroed scratch tile until then.
                wsrc = wpool.tile([P, 4 * P], BF16, tag="wsrc")
                nc.vector.memset(wsrc[:], 0.0)
                wsnk = wpool.tile([P, 4], F32, tag="wsnk")
                psw = psB.tile([P, CW], F32, tag="ps", name="psw")
                for i in range(16):
                    nc.tensor.matmul(psw[:, :HW], wsrc[:, :P], wsrc[:])
                nc.vector.tensor_copy(wsnk[:], psw[:, :4])

                # ---- L1: 16 DoubleRow fp8 matmuls per batch half.
                psh1 = psA.tile([H, CW], F32, tag="psA", name="psh1")
                for kk in range(n_s // 2):
                    for h in range(2):
                        nc.tensor.matmul(
                            psh1[:, h * HW:(h + 1) * HW], W1dr(kk), xdr(kk, h),
                            start=(kk == 0), stop=(kk == n_s // 2 - 1),
                            perf_mode=DR,
                        )
                h1T = actp.tile([H, CW], BF16, tag="act", name="h1T")
                # scale undoes the x64 put on W1 to lift fp8 denormals
                nc.scalar.activation(
                    h1T[:], psh1[:], mybir.ActivationFunctionType.Relu,
                    bias=b1s[:], scale=1.0 / W1_SCALE,
                )
                psh2 = psA.tile([H, CW], F32, tag="psA", name="psh2")
                for h in range(2):
                    nc.tensor.matmul(psh2[:, h * HW:(h + 1) * HW], W2s[:],
                                     h1T[:, h * HW:(h + 1) * HW])
                h2T = actp.tile([H, CW], BF16, tag="act", name="h2T")
                nc.scalar.activation(
                    h2T[:], psh2[:], mybir.ActivationFunctionType.Relu,
                    bias=b2s[:],
                )

                # ---- t-loop: conv + W3 matmuls (512-wide halves), then
                # 1024-wide tanh ACT and DVE add, 2-tile merged stores.
                for m in range(n_t):
                    jlist = [j for j in range(-(nb - 1), 1) if m + j >= 0]
                    psc = psB.tile([P, CW], F32, tag="ps", name=f"psc{m}")
                    for h in range(2):
                        for i, j in enumerate(jlist):
                            jj = j + nb - 1
                            nc.tensor.matmul(
                                psc[:, h * HW:(h + 1) * HW],
                                Hbs[:, jj * P:(jj + 1) * P], nt(m + j, h),
                                start=(i == 0), stop=(i == len(jlist) - 1),
                            )
                    psm = psA.tile([P, CW], F32, tag="psA", name=f"psm{m}")
                    for h in range(2):
                        nc.tensor.matmul(
                            psm[:, h * HW:(h + 1) * HW],
                            W3s[:, m * P:(m + 1) * P],
                            h2T[:, h * HW:(h + 1) * HW],
                        )
                    th = thp.tile([P, CW], BF16, tag="th")
                    nc.scalar.activation(
                        th[:], psm[:], mybir.ActivationFunctionType.Tanh,
                        bias=b3s[:, m:m + 1],
                    )
                    if m % 2 == 0:
                        ot = outp.tile([P, 2, CW], BF16, tag="ot",
                                       name=f"ot{m // 2}")
                    nc.vector.tensor_add(ot[:, m % 2, :], th[:], psc[:])
                    if m % 2 == 1:
                        dst = out_d[(m - 1) * P:(m + 1) * P, :].rearrange(
                            "(blk p) f -> p blk f", p=P)
                        nc.gpsimd.dma_start(dst, ot[:])

    nc.compile()
    return nc


# ---------------------------------------------------------------------------
# Self-contained kernel() entry point (the graded contract).
# ---------------------------------------------------------------------------

N_CORES = 8
_B, _S_IN, _S_OUT, _H, _AR = 8192, 4096, 4096, 128, 7

_CACHE = {}


def _prep_and_build(inputs):
    dev, nb = host_prepare(
        np.asarray(inputs["W1"], np.float32), np.asarray(inputs["b1"], np.float32),
        np.asarray(inputs["W2"], np.float32), np.asarray(inputs["b2"], np.float32),
        np.asarray(inputs["W3"], np.float32), np.asarray(inputs["b3"], np.float32),
        np.asarray(inputs["ar_coef"], np.float32),
        _S_IN, _S_OUT, _H,
    )
    B_total = inputs["x"].shape[0]
    B_shard = B_total // N_CORES
    key = (B_shard, nb)
    if key not in _CACHE:
        _CACHE[key] = build_kernel(B_shard, _S_IN, _S_OUT, _H, nb)
    return _CACHE[key], dev, B_shard


def _in_maps(inputs, dev, B_shard):
    x = np.asarray(inputs["x"], np.float32)
    noise_m = np.asarray(inputs["noise"], np.float32).copy()
    noise_m[:, :_AR] = 0.0
    maps = []
    for c in range(N_CORES):
        sl = slice(c * B_shard, (c + 1) * B_shard)
        m = {"xT": np.ascontiguousarray(x[sl].astype(F8_NP).T),
             "nT": np.ascontiguousarray(noise_m[sl].astype(BF16_NP).T)}
        m.update(dev)
        maps.append(m)
    return maps


def kernel(**inputs):
    nc, dev, B_shard = _prep_and_build(inputs)
    maps = _in_maps(inputs, dev, B_shard)
    import concourse.bass_utils as bass_utils

    res = bass_utils.run_bass_kernel_spmd(
        nc, maps, core_ids=list(range(N_CORES)), trace=False
    )
    shards = []
    for c in range(N_CORES):
        o = np.asarray(res.results[c]["outT"])    # [S_OUT, B_shard] bf16
        shards.append(o.T)
    return np.concatenate(shards, axis=0).astype(np.float32)


def run_traced(inputs):
    """Profiled run (NTFF -> neuron-profile) for the local test harness."""
    import contextlib
    import ctypes
    import sys as _sys
    import types as _types

    so = "/opt/axon/libaxon_pjrt.so"
    if "antenv.axon_hooks" not in _sys.modules:
        try:
            lib2 = ctypes.CDLL(so)
            lib2.axon_start_nrt_profile.argtypes = [
                ctypes.POINTER(ctypes.c_int64), ctypes.c_size_t]
            lib2.axon_start_nrt_profile.restype = ctypes.c_int64
            lib2.axon_stop_nrt_profile.argtypes = [ctypes.c_char_p]
            lib2.axon_stop_nrt_profile.restype = ctypes.c_int64

            @contextlib.contextmanager
            def _hook(output_dir, device_ids):
                import jax
                jax.devices()
                if device_ids:
                    ids_arr = (ctypes.c_int64 * len(device_ids))(*device_ids)
                    rc = lib2.axon_start_nrt_profile(ids_arr, len(device_ids))
                else:
                    rc = lib2.axon_start_nrt_profile(None, 0)
                if rc != 0:
                    raise RuntimeError(f"axon_start_nrt_profile rc={rc}")
                try:
                    yield
                finally:
                    lib2.axon_stop_nrt_profile(str(output_dir).encode())

            mod = _types.ModuleType("antenv.axon_hooks")
            mod.get_axon_ntff_profile_hook = lambda: _hook
            mod.set_axon_ntff_profile_hook = lambda h: None
            _sys.modules["antenv.axon_hooks"] = mod
        except OSError:
            pass
    import concourse.bass_utils as bass_utils
    bass_utils.upload_artifacts = lambda tmpdir: tmpdir

    nc, dev, B_shard = _prep_and_build(inputs)
    maps = _in_maps(inputs, dev, B_shard)
    return bass_utils.run_bass_kernel_spmd(
        nc, maps, core_ids=list(range(N_CORES)), trace=True, trace_cores=[0]
    )


# revision 9
# speedup vs baseline: 1.1002x; 1.1002x over previous
"""ARGenerator TRN2 kernel (v2: single 1024-wide chunk).

Math (per batch row b):
  h1 = relu(x @ W1.T + b1); h2 = relu(h1 @ W2.T + b2)
  mlp = tanh(h2 @ W3.T + b3)
  ar[t] = noise[t] + sum_i c[i] * ar[t-1-i]  (zero-init, t >= 7; 0 for t < 7)
  out = mlp + ar

The AR recurrence is linear time-invariant -> ar = conv(noise_masked, h)
with h the (geometrically decaying) impulse response, truncated at
(nb-1)*128 taps.  The conv becomes nb banded 128x128 Toeplitz matmuls
per output time-tile, fully parallel over time.

Layout strategy (pure data parallel over 8 cores, B_shard = 1024):
  bf16 everywhere (fp8 for x/W1), TRANSPOSED activation layout
  [feature/time on partitions, batch on free dim].  The host
  pre-transposes x and noise shards; output comes back transposed.

v2 pipeline: ONE chunk of CW=1024 (the full shard).  Matmuls write
512-wide PSUM bank halves (HW: matmul cannot cross a PSUM bank), but
the scalar ACT (tanh + per-partition b3 bias) and the DVE add read the
full [128, 1024] tile in ONE instruction, amortizing the (N+352)/1.2ns
fixed overhead: 1147ns per t-tile instead of 2x720.  L1 uses
DoubleRow fp8 matmuls (2 k-tiles per instruction, ~1.44x).  PSUM is
exactly 2 pools x 2 bufs x 2 banks = 8 banks.

Engine budget per t-tile (1024 batch): Tensor 6x216=1296ns,
Scalar 1147, DVE 1147, wire (noise in + out) 1312ns -> wire-paced.
Per-core wire: 0.5MB W1 + 1.1MB consts + 4MB xT + 8MB nT + 8MB out.
"""

import numpy as np
import ml_dtypes

import concourse.bass as bass
import concourse.tile as tile
import concourse.mybir as mybir
from concourse import bacc

F32 = mybir.dt.float32
BF16 = mybir.dt.bfloat16
F8 = mybir.dt.float8e4
BF16_NP = ml_dtypes.bfloat16
F8_NP = ml_dtypes.float8_e4m3
W1_SCALE = 64.0
DR = mybir.MatmulPerfMode.DoubleRow


def impulse_response(c, s_out, tail_tol=1e-4):
    """Return (h, nb) with (nb-1)*128 taps covering the response."""
    AR = len(c)
    c = np.asarray(c, np.float64)
    h = np.zeros(s_out, np.float64)
    h[0] = 1.0
    for j in range(1, s_out):
        acc = 0.0
        for i in range(AR):
            if j - 1 - i >= 0:
                acc += c[i] * h[j - 1 - i]
        h[j] = acc
    L = 128
    while L < s_out and np.abs(h[L:]).sum() > tail_tol:
        L += 128
    # nb = number of 128-wide band blocks per output tile: the in-tile block
    # (j=0) plus one per preceding input tile the L-tap history reaches into.
    return h, L // 128 + 1


def band_blocks(h, nb):
    """Hb [128, nb*128]: block jj (for input-tile offset j = jj - (nb-1))
    has Hb[k_rel, t_rel] = h[t_rel - k_rel - 128*j] (0 <= lag < (nb-1)*128)."""
    L = (nb - 1) * 128
    a = np.arange(128)[:, None]   # k_rel
    b = np.arange(128)[None, :]   # t_rel
    blocks = []
    for jj in range(nb):
        j = jj - (nb - 1)
        lag = b - a - 128 * j
        m = (lag >= 0) & (lag < L)
        blk = np.where(m, np.take(np.pad(h[:L], (0, 1)), np.clip(lag, 0, L)), 0.0)
        blocks.append(blk)
    return np.concatenate(blocks, axis=1)


def host_prepare(W1, b1, W2, b2, W3, b3, ar_coef, S_IN, S_OUT, H):
    """Small device tensors in exactly the SBUF layout used, bf16."""
    n_s = S_IN // 128
    # W1l[p, k, h] = W1[h, k*128 + p]  (lhsT tiles for layer 1)
    W1l = np.ascontiguousarray(
        W1.reshape(H, n_s, 128).transpose(2, 1, 0)
    )
    h, nb = impulse_response(ar_coef, S_OUT)
    return {
        "W1l": (W1l * W1_SCALE).astype(F8_NP),  # fp8, scaled into e4m3 normal range
        "W2l": np.ascontiguousarray(W2.T).astype(BF16_NP),   # [H_in, H_out]
        "W3l": (np.ascontiguousarray(W3.T) * W1_SCALE).astype(F8_NP),  # [H, S_OUT]
        "b1c": np.ascontiguousarray(b1.reshape(H, 1), dtype=np.float32),
        "b2c": np.ascontiguousarray(b2.reshape(H, 1), dtype=np.float32),
        "b3m": np.ascontiguousarray(b3.reshape(S_OUT // 128, 128).T,
                                    dtype=np.float32),       # [128, n_t]
        "Hb": band_blocks(h, nb).astype(BF16_NP),
    }, nb


def build_kernel(B_shard, S_IN, S_OUT, H, nb):
    P = 128
    CW = B_shard                  # 1024: one chunk, the full shard
    HW = CW // 2                  # 512: matmul free width (one PSUM bank)
    assert H == P and CW == 1024 and nb == 2
    n_s = S_IN // P               # 32 input k-tiles
    n_t = S_OUT // P              # 32 output t-tiles

    nc = bacc.Bacc(trn_type="TRN2", target_bir_lowering=False, debug=False)

    xT_d = nc.dram_tensor("xT", [S_IN, CW], F8, kind="ExternalInput").ap()
    nT_d = nc.dram_tensor("nT", [S_OUT, CW], BF16, kind="ExternalInput").ap()
    W1_d = nc.dram_tensor("W1l", [P, n_s, H], F8, kind="ExternalInput").ap()
    W2_d = nc.dram_tensor("W2l", [H, H], BF16, kind="ExternalInput").ap()
    W3_d = nc.dram_tensor("W3l", [H, S_OUT], F8, kind="ExternalInput").ap()
    b1_d = nc.dram_tensor("b1c", [H, 1], F32, kind="ExternalInput").ap()
    b2_d = nc.dram_tensor("b2c", [H, 1], F32, kind="ExternalInput").ap()
    b3_d = nc.dram_tensor("b3m", [P, n_t], F32, kind="ExternalInput").ap()
    Hb_d = nc.dram_tensor("Hb", [P, nb * P], BF16, kind="ExternalInput").ap()
    out_d = nc.dram_tensor("outT", [S_OUT, CW], BF16,
                           kind="ExternalOutput").ap()

    with tile.TileContext(nc) as tc:
        with tc.tile_pool(name="const", bufs=1) as cpool:
            # W1 split into 4 tiles across both queues: dependency tracking
            # is per-tile, so a monolithic W1 would stall layer-1 k=0 until
            # the whole 0.5MB lands.
            W1t = []
            for i in range(4):
                w = cpool.tile([P, n_s // 4, H], F8, tag=f"w1_{i}")
                (nc.sync if i % 2 == 0 else nc.scalar).dma_start(
                    w[:], W1_d[:, i * (n_s // 4):(i + 1) * (n_s // 4), :])
                W1t.append(w)

            def W1dr(kk):     # k-pair kk -> [128, 2, H] fp8 lhsT
                return W1t[kk // 4][:, 2 * (kk % 4):2 * (kk % 4) + 2, :]

            # scalar queue: small consts (x loads follow on sync+scalar).
            W2s = cpool.tile([H, H], BF16, tag="w2")
            nc.scalar.dma_start(W2s[:], W2_d[:])
            b1s = cpool.tile([H, 1], F32, tag="b1")
            nc.scalar.dma_start(b1s[:], b1_d[:])
            b2s = cpool.tile([H, 1], F32, tag="b2")
            nc.scalar.dma_start(b2s[:], b2_d[:])
            # t-loop consts on the gpsimd queue (idle until stores begin).
            b3s = cpool.tile([P, n_t], F32, tag="b3")
            nc.gpsimd.dma_start(b3s[:], b3_d[:])
            Hbs = cpool.tile([P, nb * P], BF16, tag="hb")
            nc.gpsimd.dma_start(Hbs[:], Hb_d[:])
            W3s = cpool.tile([H, S_OUT], F8, tag="w3")
            nc.gpsimd.dma_start(W3s[:], W3_d[:])

            with (
                tc.tile_pool(name="warm", bufs=1) as wpool,
                tc.tile_pool(name="xT", bufs=8) as xTp,
                tc.tile_pool(name="nT", bufs=8) as nTp,
                tc.tile_pool(name="act", bufs=2) as actp,
                tc.tile_pool(name="th", bufs=3) as thp,
                tc.tile_pool(name="outT", bufs=3) as outp,
                tc.tile_pool(name="psA", bufs=2, space="PSUM") as psA,
                tc.tile_pool(name="psB", bufs=2, space="PSUM") as psB,
            ):
                # ---- input loads: 4 k-tiles per DMA (512KB for x, 1MB for
                # noise).  Per-queue descriptor FIFO is the wire-priority
                # mechanism: ALL x groups are issued before ALL noise groups
                # on each of the two queues, so layer 1 is never starved by
                # noise traffic.  bufs=8 on both pools (everything resident)
                # so no trigger ever blocks an engine queue ahead of the
                # ACTs that share it.
                xts = []
                for g in range(n_s // 4):
                    t = xTp.tile([P, 4, CW], F8, tag="xt", name=f"xt{g}")
                    src = xT_d[g * 4 * P:(g + 1) * 4 * P, :].rearrange(
                        "(blk p) f -> p blk f", p=P)
                    (nc.sync if g % 2 == 0 else nc.scalar).dma_start(t[:], src)
                    xts.append(t)

                def xdr(kk, h):   # k-pair kk, batch-half h -> [128,2,512] f8
                    g, j = kk // 2, 2 * (kk % 2)
                    return xts[g][:, j:j + 2, h * HW:(h + 1) * HW]

                ntm = []
                for g in range(n_t // 4):
                    t = nTp.tile([P, 4, CW], BF16, tag="nt", name=f"nt{g}")
                    src = nT_d[g * 4 * P:(g + 1) * 4 * P, :].rearrange(
                        "(blk p) f -> p blk f", p=P)
                    (nc.sync if g % 2 == 0 else nc.scalar).dma_start(t[:], src)
                    ntm.append(t)

                def nt(m, h):
                    return ntm[m // 4][:, m % 4, h * HW:(h + 1) * HW]

                # ---- PE warm-up: the HAM clock gate defaults the PE array
                # to 1.2 GHz and only releases 2.4 GHz after ~3.4us of
                # sustained matmul activity; it re-throttles after ~3.4us
                # idle.  The first real matmul cannot start until W1+x
                # arrive (~6us); warm on a zeroed scratch tile until then.
                wsrc = wpool.tile([P, 4 * P], BF16, tag="wsrc")
                nc.vector.memset(wsrc[:], 0.0)
                wsnk = wpool.tile([P, 4], F32, tag="wsnk")
                psw = psB.tile([P, CW], F32, tag="ps", name="psw")
                for i in range(16):
                    nc.tensor.matmul(psw[:, :HW], wsrc[:, :P], wsrc[:])
                nc.vector.tensor_copy(wsnk[:], psw[:, :4])

                # ---- L1: 16 DoubleRow fp8 matmuls per batch half.
                psh1 = psA.tile([H, CW], F32, tag="psA", name="psh1")
                for kk in range(n_s // 2):
                    for h in range(2):
                        nc.tensor.matmul(
                            psh1[:, h * HW:(h + 1) * HW], W1dr(kk), xdr(kk, h),
                            start=(kk == 0), stop=(kk == n_s // 2 - 1),
                            perf_mode=DR,
                        )
                h1T = actp.tile([H, CW], BF16, tag="act", name="h1T")
                # scale undoes the x64 put on W1 to lift fp8 denormals
                nc.scalar.activation(
                    h1T[:], psh1[:], mybir.ActivationFunctionType.Relu,
                    bias=b1s[:], scale=1.0 / W1_SCALE,
                )
                psh2 = psA.tile([H, CW], F32, tag="psA", name="psh2")
                for h in range(2):
                    nc.tensor.matmul(psh2[:, h * HW:(h + 1) * HW], W2s[:],
                                     h1T[:, h * HW:(h + 1) * HW])
                h2T = actp.tile([H, CW], BF16, tag="act", name="h2T")
                nc.scalar.activation(
                    h2T[:], psh2[:], mybir.ActivationFunctionType.Relu,
                    bias=b2s[:],
                )

                # ---- t-loop: conv + W3 matmuls (512-wide halves, grouped by
                # stationary so LDWEIGHTS serves both halves), then 1024-wide
                # tanh ACT and the final add (every 4th on the pool engine to
                # keep DVE under the tensor/wire pace), 2-tile merged stores.
                for m in range(n_t):
                    jlist = [j for j in range(-(nb - 1), 1) if m + j >= 0]
                    psc = psB.tile([P, CW], F32, tag="ps", name=f"psc{m}")
                    for i, j in enumerate(jlist):
                        jj = j + nb - 1
                        for h in range(2):
                            nc.tensor.matmul(
                                psc[:, h * HW:(h + 1) * HW],
                                Hbs[:, jj * P:(jj + 1) * P], nt(m + j, h),
                                start=(i == 0), stop=(i == len(jlist) - 1),
                            )
                    psm = psA.tile([P, CW], F32, tag="psA", name=f"psm{m}")
                    for h in range(2):
                        nc.tensor.matmul(
                            psm[:, h * HW:(h + 1) * HW],
                            W3s[:, m * P:(m + 1) * P],
                            h2T[:, h * HW:(h + 1) * HW],
                        )
                    if m % 2 == 0:
                        ot = outp.tile([P, 2, CW], BF16, tag="ot",
                                       name=f"ot{m // 2}")
                    # (GPSIMD/Pool cannot read PSUM -> all adds on DVE)
                    add_eng = nc.vector
                    if m == n_t - 1:
                        # finer-grained tail: per-half ACT/add/store so the
                        # drain after the last matmul is ~2us shorter.
                        th = thp.tile([P, CW], BF16, tag="th")
                        for h in range(2):
                            hs = slice(h * HW, (h + 1) * HW)
                            nc.scalar.activation(
                                th[:, hs], psm[:, hs],
                                mybir.ActivationFunctionType.Tanh,
                                bias=b3s[:, m:m + 1], scale=1.0 / W1_SCALE,
                            )
                            add_eng.tensor_add(ot[:, 1, hs], th[:, hs],
                                               psc[:, hs])
                            dst = out_d[(m - 1) * P:(m + 1) * P,
                                        h * HW:(h + 1) * HW].rearrange(
                                "(blk p) f -> p blk f", p=P)
                            nc.gpsimd.dma_start(dst, ot[:, :, hs])
                        continue
                    th = thp.tile([P, CW], BF16, tag="th")
                    nc.scalar.activation(
                        th[:], psm[:], mybir.ActivationFunctionType.Tanh,
                        bias=b3s[:, m:m + 1], scale=1.0 / W1_SCALE,
                    )
                    add_eng.tensor_add(ot[:, m % 2, :], th[:], psc[:])
                    if m % 2 == 1:
                        dst = out_d[(m - 1) * P:(m + 1) * P, :].rearrange(
                            "(blk p) f -> p blk f", p=P)
                        nc.gpsimd.dma_start(dst, ot[:])

    nc.compile()
    return nc


# ---------------------------------------------------------------------------
# Self-contained kernel() entry point (the graded contract).
# ---------------------------------------------------------------------------

N_CORES = 8
_B, _S_IN, _S_OUT, _H, _AR = 8192, 4096, 4096, 128, 7

_CACHE = {}


def _prep_and_build(inputs):
    dev, nb = host_prepare(
        np.asarray(inputs["W1"], np.float32), np.asarray(inputs["b1"], np.float32),
        np.asarray(inputs["W2"], np.float32), np.asarray(inputs["b2"], np.float32),
        np.asarray(inputs["W3"], np.float32), np.asarray(inputs["b3"], np.float32),
        np.asarray(inputs["ar_coef"], np.float32),
        _S_IN, _S_OUT, _H,
    )
    B_total = inputs["x"].shape[0]
    B_shard = B_total // N_CORES
    key = (B_shard, nb)
    if key not in _CACHE:
        _CACHE[key] = build_kernel(B_shard, _S_IN, _S_OUT, _H, nb)
    return _CACHE[key], dev, B_shard


def _in_maps(inputs, dev, B_shard):
    x = np.asarray(inputs["x"], np.float32)
    noise_m = np.asarray(inputs["noise"], np.float32).copy()
    noise_m[:, :_AR] = 0.0
    maps = []
    for c in range(N_CORES):
        sl = slice(c * B_shard, (c + 1) * B_shard)
        m = {"xT": np.ascontiguousarray(x[sl].astype(F8_NP).T),
             "nT": np.ascontiguousarray(noise_m[sl].astype(BF16_NP).T)}
        m.update(dev)
        maps.append(m)
    return maps


def kernel(**inputs):
    nc, dev, B_shard = _prep_and_build(inputs)
    maps = _in_maps(inputs, dev, B_shard)
    import concourse.bass_utils as bass_utils

    res = bass_utils.run_bass_kernel_spmd(
        nc, maps, core_ids=list(range(N_CORES)), trace=False
    )
    shards = []
    for c in range(N_CORES):
        o = np.asarray(res.results[c]["outT"])    # [S_OUT, B_shard] bf16
        shards.append(o.T)
    return np.concatenate(shards, axis=0).astype(np.float32)


def run_traced(inputs):
    """Profiled run (NTFF -> neuron-profile) for the local test harness."""
    import contextlib
    import ctypes
    import sys as _sys
    import types as _types

    so = "/opt/axon/libaxon_pjrt.so"
    if "antenv.axon_hooks" not in _sys.modules:
        try:
            lib2 = ctypes.CDLL(so)
            lib2.axon_start_nrt_profile.argtypes = [
                ctypes.POINTER(ctypes.c_int64), ctypes.c_size_t]
            lib2.axon_start_nrt_profile.restype = ctypes.c_int64
            lib2.axon_stop_nrt_profile.argtypes = [ctypes.c_char_p]
            lib2.axon_stop_nrt_profile.restype = ctypes.c_int64

            @contextlib.contextmanager
            def _hook(output_dir, device_ids):
                import jax
                jax.devices()
                if device_ids:
                    ids_arr = (ctypes.c_int64 * len(device_ids))(*device_ids)
                    rc = lib2.axon_start_nrt_profile(ids_arr, len(device_ids))
                else:
                    rc = lib2.axon_start_nrt_profile(None, 0)
                if rc != 0:
                    raise RuntimeError(f"axon_start_nrt_profile rc={rc}")
                try:
                    yield
                finally:
                    lib2.axon_stop_nrt_profile(str(output_dir).encode())

            mod = _types.ModuleType("antenv.axon_hooks")
            mod.get_axon_ntff_profile_hook = lambda: _hook
            mod.set_axon_ntff_profile_hook = lambda h: None
            _sys.modules["antenv.axon_hooks"] = mod
        except OSError:
            pass
    import concourse.bass_utils as bass_utils
    bass_utils.upload_artifacts = lambda tmpdir: tmpdir

    nc, dev, B_shard = _prep_and_build(inputs)
    maps = _in_maps(inputs, dev, B_shard)
    return bass_utils.run_bass_kernel_spmd(
        nc, maps, core_ids=list(range(N_CORES)), trace=True, trace_cores=[0]
    )


# revision 11
# speedup vs baseline: 1.1331x; 1.0300x over previous
"""ARGenerator TRN2 kernel (v2: single 1024-wide chunk).

Math (per batch row b):
  h1 = relu(x @ W1.T + b1); h2 = relu(h1 @ W2.T + b2)
  mlp = tanh(h2 @ W3.T + b3)
  ar[t] = noise[t] + sum_i c[i] * ar[t-1-i]  (zero-init, t >= 7; 0 for t < 7)
  out = mlp + ar

The AR recurrence is linear time-invariant -> ar = conv(noise_masked, h)
with h the (geometrically decaying) impulse response, truncated at
(nb-1)*128 taps.  The conv becomes nb banded 128x128 Toeplitz matmuls
per output time-tile, fully parallel over time.

Layout strategy (pure data parallel over 8 cores, B_shard = 1024):
  bf16 everywhere (fp8 for x/W1), TRANSPOSED activation layout
  [feature/time on partitions, batch on free dim].  The host
  pre-transposes x and noise shards; output comes back transposed.

v2 pipeline: ONE chunk of CW=1024 (the full shard).  Matmuls write
512-wide PSUM bank halves (HW: matmul cannot cross a PSUM bank), but
the scalar ACT (tanh + per-partition b3 bias) and the DVE add read the
full [128, 1024] tile in ONE instruction, amortizing the (N+352)/1.2ns
fixed overhead: 1147ns per t-tile instead of 2x720.  L1 uses
DoubleRow fp8 matmuls (2 k-tiles per instruction, ~1.44x).  PSUM is
exactly 2 pools x 2 bufs x 2 banks = 8 banks.

Engine budget per t-tile (1024 batch): Tensor 6x216=1296ns,
Scalar 1147, DVE 1147, wire (noise in + out) 1312ns -> wire-paced.
Per-core wire: 0.5MB W1 + 1.1MB consts + 4MB xT + 8MB nT + 8MB out.
"""

import numpy as np
import ml_dtypes

import concourse.bass as bass
import concourse.tile as tile
import concourse.mybir as mybir
from concourse import bacc

F32 = mybir.dt.float32
BF16 = mybir.dt.bfloat16
F8 = mybir.dt.float8e4
BF16_NP = ml_dtypes.bfloat16
F8_NP = ml_dtypes.float8_e4m3
W1_SCALE = 64.0
DR = mybir.MatmulPerfMode.DoubleRow


def impulse_response(c, s_out, tail_tol=1e-4):
    """Return (h, nb) with (nb-1)*128 taps covering the response."""
    AR = len(c)
    c = np.asarray(c, np.float64)
    h = np.zeros(s_out, np.float64)
    h[0] = 1.0
    for j in range(1, s_out):
        acc = 0.0
        for i in range(AR):
            if j - 1 - i >= 0:
                acc += c[i] * h[j - 1 - i]
        h[j] = acc
    L = 128
    while L < s_out and np.abs(h[L:]).sum() > tail_tol:
        L += 128
    # nb = number of 128-wide band blocks per output tile: the in-tile block
    # (j=0) plus one per preceding input tile the L-tap history reaches into.
    return h, L // 128 + 1


def band_blocks(h, nb):
    """Hb [128, nb*128]: block jj (for input-tile offset j = jj - (nb-1))
    has Hb[k_rel, t_rel] = h[t_rel - k_rel - 128*j] (0 <= lag < (nb-1)*128)."""
    L = (nb - 1) * 128
    a = np.arange(128)[:, None]   # k_rel
    b = np.arange(128)[None, :]   # t_rel
    blocks = []
    for jj in range(nb):
        j = jj - (nb - 1)
        lag = b - a - 128 * j
        m = (lag >= 0) & (lag < L)
        blk = np.where(m, np.take(np.pad(h[:L], (0, 1)), np.clip(lag, 0, L)), 0.0)
        blocks.append(blk)
    return np.concatenate(blocks, axis=1)


def host_prepare(W1, b1, W2, b2, W3, b3, ar_coef, S_IN, S_OUT, H):
    """Small device tensors in exactly the SBUF layout used, bf16."""
    n_s = S_IN // 128
    # W1l[p, k, h] = W1[h, k*128 + p]  (lhsT tiles for layer 1)
    W1l = np.ascontiguousarray(
        W1.reshape(H, n_s, 128).transpose(2, 1, 0)
    )
    h, nb = impulse_response(ar_coef, S_OUT)
    b3m = b3.reshape(S_OUT // 128, 128).T                    # [128, n_t]
    # merge the small consts into two blobs (one descriptor each): many
    # small DMAs serialize on their completion semaphores and drag the
    # critical load prefix.
    wb = np.concatenate(
        [np.ascontiguousarray(W2.T), band_blocks(h, nb)], axis=1)
    bias = np.concatenate(
        [b1.reshape(H, 1), b2.reshape(H, 1), b3m], axis=1)
    return {
        "W1l": (W1l * W1_SCALE).astype(F8_NP),  # fp8, scaled into e4m3 normal range
        "W3l": (np.ascontiguousarray(W3.T) * W1_SCALE).astype(F8_NP),  # [H, S_OUT]
        "WbB": wb.astype(BF16_NP),              # [128, H + nb*128]: W2l | Hb
        "bias": np.ascontiguousarray(bias, np.float32),  # [128, 2+n_t]
    }, nb


def build_kernel(B_shard, S_IN, S_OUT, H, nb):
    P = 128
    CW = B_shard                  # 1024: one chunk, the full shard
    HW = CW // 2                  # 512: matmul free width (one PSUM bank)
    assert H == P and CW == 1024 and nb == 2
    n_s = S_IN // P               # 32 input k-tiles
    n_t = S_OUT // P              # 32 output t-tiles

    nc = bacc.Bacc(trn_type="TRN2", target_bir_lowering=False, debug=False)

    xT_d = nc.dram_tensor("xT", [S_IN, CW], F8, kind="ExternalInput").ap()
    nT_d = nc.dram_tensor("nT", [S_OUT, CW], BF16, kind="ExternalInput").ap()
    W1_d = nc.dram_tensor("W1l", [P, n_s, H], F8, kind="ExternalInput").ap()
    W3_d = nc.dram_tensor("W3l", [H, S_OUT], F8, kind="ExternalInput").ap()
    Wb_d = nc.dram_tensor("WbB", [P, H + nb * P], BF16,
                          kind="ExternalInput").ap()
    bias_d = nc.dram_tensor("bias", [P, 2 + n_t], F32,
                            kind="ExternalInput").ap()
    out_d = nc.dram_tensor("outT", [S_OUT, CW], BF16,
                           kind="ExternalOutput").ap()

    with tile.TileContext(nc) as tc:
        with tc.tile_pool(name="const", bufs=1) as cpool:
            # DMA triggers serialize on their completion semaphores (the
            # next trigger on a semaphore waits for the previous transfer),
            # so per-queue throughput ~ descriptor_size/(dma+trigger).  Use
            # 1MB descriptors and keep the queues busy: W1 + half of x on
            # scalar, half of x then ALL noise on sync (per-queue FIFO keeps
            # x ahead of noise on the wire), consts + W3 + stores on gpsimd.
            W1s = cpool.tile([P, n_s, H], F8, tag="w1")
            nc.scalar.dma_start(W1s[:], W1_d[:])

            def W1dr(kk):     # k-pair kk -> [128, 2, H] fp8 lhsT
                return W1s[:, 2 * kk:2 * kk + 2, :]

            Wbs = cpool.tile([P, H + nb * P], BF16, tag="wb")
            nc.gpsimd.dma_start(Wbs[:], Wb_d[:])
            W2s = Wbs[:, :H]
            Hbs = Wbs[:, H:]
            biass = cpool.tile([P, 2 + n_t], F32, tag="bias")
            nc.gpsimd.dma_start(biass[:], bias_d[:])
            b1s = biass[:, 0:1]
            b2s = biass[:, 1:2]
            b3s = biass[:, 2:]
            W3s = cpool.tile([H, S_OUT], F8, tag="w3")
            nc.gpsimd.dma_start(W3s[:], W3_d[:])

            with (
                tc.tile_pool(name="warm", bufs=1) as wpool,
                tc.tile_pool(name="xT", bufs=4) as xTp,
                tc.tile_pool(name="nT", bufs=8) as nTp,
                tc.tile_pool(name="act", bufs=2) as actp,
                tc.tile_pool(name="th", bufs=3) as thp,
                tc.tile_pool(name="outT", bufs=3) as outp,
                tc.tile_pool(name="psA", bufs=2, space="PSUM") as psA,
                tc.tile_pool(name="psB", bufs=2, space="PSUM") as psB,
            ):
                # ---- x: 4 descriptors of 1MB (8 k-tiles each), alternating
                # sync/scalar so two DMA chains run in parallel.
                xts = []
                for g in range(n_s // 8):
                    t = xTp.tile([P, 8, CW], F8, tag="xt", name=f"xt{g}")
                    src = xT_d[g * 8 * P:(g + 1) * 8 * P, :].rearrange(
                        "(blk p) f -> p blk f", p=P)
                    (nc.sync if g % 2 == 0 else nc.scalar).dma_start(t[:], src)
                    xts.append(t)

                def xdr(kk, h):   # k-pair kk, batch-half h -> [128,2,512] f8
                    g, j = kk // 4, 2 * (kk % 4)
                    return xts[g][:, j:j + 2, h * HW:(h + 1) * HW]

                # ---- noise: 8 descriptors of 1MB, all on sync (it has
                # nothing else to do; scalar must stay clear for the ACTs).
                ntm = []
                for g in range(n_t // 4):
                    t = nTp.tile([P, 4, CW], BF16, tag="nt", name=f"nt{g}")
                    src = nT_d[g * 4 * P:(g + 1) * 4 * P, :].rearrange(
                        "(blk p) f -> p blk f", p=P)
                    nc.sync.dma_start(t[:], src)
                    ntm.append(t)

                def nt(m, h):
                    return ntm[m // 4][:, m % 4, h * HW:(h + 1) * HW]

                # ---- PE warm-up: the HAM clock gate defaults the PE array
                # to 1.2 GHz and only releases 2.4 GHz after ~3.4us of
                # sustained matmul activity; it re-throttles after ~3.4us
                # idle.  The first real matmul cannot start until W1+x
                # arrive (~6us); warm on a zeroed scratch tile until then.
                wsrc = wpool.tile([P, 4 * P], BF16, tag="wsrc")
                nc.vector.memset(wsrc[:], 0.0)
                wsnk = wpool.tile([P, 4], F32, tag="wsnk")
                psw = psB.tile([P, CW], F32, tag="ps", name="psw")
                for i in range(16):
                    nc.tensor.matmul(psw[:, :HW], wsrc[:, :P], wsrc[:])
                nc.vector.tensor_copy(wsnk[:], psw[:, :4])

                # ---- L1: 16 DoubleRow fp8 matmuls per batch half.
                psh1 = psA.tile([H, CW], F32, tag="psA", name="psh1")
                for kk in range(n_s // 2):
                    for h in range(2):
                        nc.tensor.matmul(
                            psh1[:, h * HW:(h + 1) * HW], W1dr(kk), xdr(kk, h),
                            start=(kk == 0), stop=(kk == n_s // 2 - 1),
                            perf_mode=DR,
                        )
                h1T = actp.tile([H, CW], BF16, tag="act", name="h1T")
                # scale undoes the x64 put on W1 to lift fp8 denormals
                nc.scalar.activation(
                    h1T[:], psh1[:], mybir.ActivationFunctionType.Relu,
                    bias=b1s[:], scale=1.0 / W1_SCALE,
                )
                psh2 = psA.tile([H, CW], F32, tag="psA", name="psh2")
                for h in range(2):
                    nc.tensor.matmul(psh2[:, h * HW:(h + 1) * HW], W2s[:],
                                     h1T[:, h * HW:(h + 1) * HW])
                h2T = actp.tile([H, CW], BF16, tag="act", name="h2T")
                nc.scalar.activation(
                    h2T[:], psh2[:], mybir.ActivationFunctionType.Relu,
                    bias=b2s[:],
                )

                # ---- t-loop: conv + W3 matmuls (512-wide halves, grouped by
                # stationary so LDWEIGHTS serves both halves), then 1024-wide
                # tanh ACT and the final add (every 4th on the pool engine to
                # keep DVE under the tensor/wire pace), 2-tile merged stores.
                for m in range(n_t):
                    jlist = [j for j in range(-(nb - 1), 1) if m + j >= 0]
                    psc = psB.tile([P, CW], F32, tag="ps", name=f"psc{m}")
                    for i, j in enumerate(jlist):
                        jj = j + nb - 1
                        for h in range(2):
                            nc.tensor.matmul(
                                psc[:, h * HW:(h + 1) * HW],
                                Hbs[:, jj * P:(jj + 1) * P], nt(m + j, h),
                                start=(i == 0), stop=(i == len(jlist) - 1),
                            )
                    psm = psA.tile([P, CW], F32, tag="psA", name=f"psm{m}")
                    for h in range(2):
                        nc.tensor.matmul(
                            psm[:, h * HW:(h + 1) * HW],
                            W3s[:, m * P:(m + 1) * P],
                            h2T[:, h * HW:(h + 1) * HW],
                        )
                    if m % 2 == 0:
                        ot = outp.tile([P, 2, CW], BF16, tag="ot",
                                       name=f"ot{m // 2}")
                    # (GPSIMD/Pool cannot read PSUM -> all adds on DVE)
                    add_eng = nc.vector
                    if m == n_t - 1:
                        # finer-grained tail: per-half ACT/add/store so the
                        # drain after the last matmul is ~2us shorter.
                        th = thp.tile([P, CW], BF16, tag="th")
                        for h in range(2):
                            hs = slice(h * HW, (h + 1) * HW)
                            nc.scalar.activation(
                                th[:, hs], psm[:, hs],
                                mybir.ActivationFunctionType.Tanh,
                                bias=b3s[:, m:m + 1], scale=1.0 / W1_SCALE,
                            )
                            add_eng.tensor_add(ot[:, 1, hs], th[:, hs],
                                               psc[:, hs])
                            dst = out_d[(m - 1) * P:(m + 1) * P,
                                        h * HW:(h + 1) * HW].rearrange(
                                "(blk p) f -> p blk f", p=P)
                            nc.gpsimd.dma_start(dst, ot[:, :, hs])
                        continue
                    th = thp.tile([P, CW], BF16, tag="th")
                    nc.scalar.activation(
                        th[:], psm[:], mybir.ActivationFunctionType.Tanh,
                        bias=b3s[:, m:m + 1], scale=1.0 / W1_SCALE,
                    )
                    add_eng.tensor_add(ot[:, m % 2, :], th[:], psc[:])
                    if m % 2 == 1:
                        dst = out_d[(m - 1) * P:(m + 1) * P, :].rearrange(
                            "(blk p) f -> p blk f", p=P)
                        nc.gpsimd.dma_start(dst, ot[:])

    nc.compile()
    return nc


# ---------------------------------------------------------------------------
# Self-contained kernel() entry point (the graded contract).
# ---------------------------------------------------------------------------

N_CORES = 8
_B, _S_IN, _S_OUT, _H, _AR = 8192, 4096, 4096, 128, 7

_CACHE = {}


def _prep_and_build(inputs):
    dev, nb = host_prepare(
        np.asarray(inputs["W1"], np.float32), np.asarray(inputs["b1"], np.float32),
        np.asarray(inputs["W2"], np.float32), np.asarray(inputs["b2"], np.float32),
        np.asarray(inputs["W3"], np.float32), np.asarray(inputs["b3"], np.float32),
        np.asarray(inputs["ar_coef"], np.float32),
        _S_IN, _S_OUT, _H,
    )
    B_total = inputs["x"].shape[0]
    B_shard = B_total // N_CORES
    key = (B_shard, nb)
    if key not in _CACHE:
        _CACHE[key] = build_kernel(B_shard, _S_IN, _S_OUT, _H, nb)
    return _CACHE[key], dev, B_shard


def _in_maps(inputs, dev, B_shard):
    x = np.asarray(inputs["x"], np.float32)
    noise_m = np.asarray(inputs["noise"], np.float32).copy()
    noise_m[:, :_AR] = 0.0
    maps = []
    for c in range(N_CORES):
        sl = slice(c * B_shard, (c + 1) * B_shard)
        m = {"xT": np.ascontiguousarray(x[sl].astype(F8_NP).T),
             "nT": np.ascontiguousarray(noise_m[sl].astype(BF16_NP).T)}
        m.update(dev)
        maps.append(m)
    return maps


def kernel(**inputs):
    nc, dev, B_shard = _prep_and_build(inputs)
    maps = _in_maps(inputs, dev, B_shard)
    import concourse.bass_utils as bass_utils

    res = bass_utils.run_bass_kernel_spmd(
        nc, maps, core_ids=list(range(N_CORES)), trace=False
    )
    shards = []
    for c in range(N_CORES):
        o = np.asarray(res.results[c]["outT"])    # [S_OUT, B_shard] bf16
        shards.append(o.T)
    return np.concatenate(shards, axis=0).astype(np.float32)


def run_traced(inputs):
    """Profiled run (NTFF -> neuron-profile) for the local test harness."""
    import contextlib
    import ctypes
    import sys as _sys
    import types as _types

    so = "/opt/axon/libaxon_pjrt.so"
    if "antenv.axon_hooks" not in _sys.modules:
        try:
            lib2 = ctypes.CDLL(so)
            lib2.axon_start_nrt_profile.argtypes = [
                ctypes.POINTER(ctypes.c_int64), ctypes.c_size_t]
            lib2.axon_start_nrt_profile.restype = ctypes.c_int64
            lib2.axon_stop_nrt_profile.argtypes = [ctypes.c_char_p]
            lib2.axon_stop_nrt_profile.restype = ctypes.c_int64

            @contextlib.contextmanager
            def _hook(output_dir, device_ids):
                import jax
                jax.devices()
                if device_ids:
                    ids_arr = (ctypes.c_int64 * len(device_ids))(*device_ids)
                    rc = lib2.axon_start_nrt_profile(ids_arr, len(device_ids))
                else:
                    rc = lib2.axon_start_nrt_profile(None, 0)
                if rc != 0:
                    raise RuntimeError(f"axon_start_nrt_profile rc={rc}")
                try:
                    yield
                finally:
                    lib2.axon_stop_nrt_profile(str(output_dir).encode())

            mod = _types.ModuleType("antenv.axon_hooks")
            mod.get_axon_ntff_profile_hook = lambda: _hook
            mod.set_axon_ntff_profile_hook = lambda h: None
            _sys.modules["antenv.axon_hooks"] = mod
        except OSError:
            pass
    import concourse.bass_utils as bass_utils
    bass_utils.upload_artifacts = lambda tmpdir: tmpdir

    nc, dev, B_shard = _prep_and_build(inputs)
    maps = _in_maps(inputs, dev, B_shard)
    return bass_utils.run_bass_kernel_spmd(
        nc, maps, core_ids=list(range(N_CORES)), trace=True, trace_cores=[0]
    )
